# revision 1
# baseline (speedup 1.0000x reference)
"""Spiking transformer block (nn_Block_22170621182450) on 8 trn2 NeuronCores.

Data-parallel over B (2 batch elems/core). Channel-major on-chip layout
[C_out, tokens]; tokens are t-major so LIF time slabs are contiguous.
BN statistics are globalized with tiny AllReduces (sum, sumsq per channel).

Precision plan (validated by numpy flip-simulation against the exact ref):
  - q/k/v/p linears: single-pass float32r (TF32-like, 11 mantissa bits).
    The attention path is exact integer arithmetic and the o-LIF membrane
    lives on a 1/16 grid, which absorbs all qkv/p-level rounding noise.
  - f1: 3-pass float32r hi/lo split of both operands (~23 mantissa bits).
  - f2: 2-pass bf16 hi/lo split of the weight (activations are binary).
  - attention: bf16 (binary spikes / small integers -> exact).
Linear biases are dropped entirely: BatchNorm removes per-channel shifts.
"""

import os
import sys

for p in ("/opt/trn_rl_repo", "/root/.axon_site", "/root/.axon_site/_ro/trn_rl_repo",
          "/root/.axon_site/_ro/pypackages"):
    if os.path.isdir(p) and p not in sys.path:
        sys.path.append(p)

import numpy as np
import ml_dtypes

from contextlib import ExitStack
import concourse.bass as bass
import concourse.bacc as bacc
import concourse.tile as tile
from concourse import mybir
from concourse import bass_utils
from concourse.dve_ops import TENSOR_MASK
from concourse.masks import make_identity

F32 = mybir.dt.float32
F32R = mybir.dt.float32r
BF16 = mybir.dt.bfloat16
AX = mybir.AluOpType
AF = mybir.ActivationFunctionType

T, B, N, C, H = 4, 16, 256, 512, 8
HID = 2048
NCORES = 8
BL = B // NCORES            # 2 batch elems per core
S = T * BL * N              # 2048 tokens per core
SLAB = BL * N               # 512 tokens per time step
S_TOT = T * B * N           # 16384 tokens globally (BN population)
CT_C = C // 128             # 4 channel tiles for C
CT_H = HID // 128           # 16 channel tiles for HID
EPS = 1e-5
SCALE = 0.125
P = 128

_CACHE = {}
UPTO = int(os.environ.get("KERNEL_PHASES", "8"))
NO_COLL = os.environ.get("KERNEL_NO_COLL", "0") == "1"


def _round_mant(x, m=11):
    """Round fp32 to m explicit mantissa bits (float32r grid)."""
    x = np.ascontiguousarray(x, np.float32)
    b = x.view(np.uint32).astype(np.uint64)
    shift = 23 - m
    add = np.uint64(1 << (shift - 1))
    mask = np.uint64(~((1 << shift) - 1) & 0xFFFFFFFF)
    return ((b + add) & mask).astype(np.uint32).view(np.float32)


def _f32r_split(x):
    hi = _round_mant(x, 11)
    lo = _round_mant(x.astype(np.float32) - hi, 11)
    return hi, lo


def _bf16_split(x):
    hi = x.astype(ml_dtypes.bfloat16)
    lo = (x.astype(np.float32) - hi.astype(np.float32)).astype(ml_dtypes.bfloat16)
    return hi, lo


def _pack_ch(v, n_ct):
    """[n_ct*128] channel vector -> [128, n_ct] (channel%128 on partitions)."""
    return np.ascontiguousarray(np.asarray(v, np.float32).reshape(n_ct, P).T)


def _build():
    nc = bacc.Bacc("TRN2", target_bir_lowering=False, debug=False,
                   num_devices=NCORES)

    def dt_in(name, shape, dtype):
        return nc.dram_tensor(name, list(shape), dtype,
                              kind="ExternalInput").ap()

    xt_hi = dt_in("xt_hi", (C, S), F32R)
    xt_lo = dt_in("xt_lo", (C, S), F32R)
    w_lin = {"q": dt_in("w_q", (C, C), F32R),
             "k": dt_in("w_k", (C, C), F32R),
             "v": dt_in("w_v", (C, C), F32R),
             "p": dt_in("w_p", (C, C), F32R)}
    w_f1h = dt_in("w_f1h", (C, HID), F32R)
    w_f1l = dt_in("w_f1l", (C, HID), F32R)
    w_f2h = dt_in("w_f2h", (HID, C), BF16)
    w_f2l = dt_in("w_f2l", (HID, C), BF16)
    gbe_d = {}
    for nm, n_ct in (("q", CT_C), ("k", CT_C), ("v", CT_C), ("p", CT_C),
                     ("f1", CT_H), ("f2", CT_C)):
        gbe_d[nm] = (dt_in(f"{nm}_gp", (P, n_ct), F32),
                     dt_in(f"{nm}_bp", (P, n_ct), F32))
    out_d = nc.dram_tensor("outT", [C, S], F32, kind="ExternalOutput").ap()

    RG = [list(range(NCORES))]

    with tile.TileContext(nc) as tc:
        with ExitStack() as _es:
            constp = _es.enter_context(tc.tile_pool(name="const", bufs=1))
            gbep = _es.enter_context(tc.tile_pool(name="gbep", bufs=1))
            xp = _es.enter_context(tc.tile_pool(name="xsplit", bufs=8))
            hp = _es.enter_context(tc.tile_pool(name="hpool", bufs=8))
            mp = _es.enter_context(tc.tile_pool(name="mstate", bufs=2))
            statp = _es.enter_context(tc.tile_pool(name="stats", bufs=4))
            bnp = _es.enter_context(tc.tile_pool(name="bnconst", bufs=4))
            scrp = _es.enter_context(tc.tile_pool(name="scratch", bufs=2))
            dramp = _es.enter_context(tc.tile_pool(name="dram", bufs=2, space="DRAM"))

            eps_t = constp.tile([P, 1], F32, tag="eps")
            nc.vector.memset(eps_t[:], EPS)
            ident = constp.tile([P, P], BF16, tag="ident")
            make_identity(nc, ident[:])

            gbe_sb = {}
            for nm in gbe_d:
                n_ct = CT_H if nm == "f1" else CT_C
                gt = gbep.tile([P, n_ct], F32, tag=f"g_{nm}")
                bt = gbep.tile([P, n_ct], F32, tag=f"b_{nm}")
                nc.sync.dma_start(out=gt[:], in_=gbe_d[nm][0])
                nc.sync.dma_start(out=bt[:], in_=gbe_d[nm][1])
                gbe_sb[nm] = (gt, bt)

            # x splits, channel-major [C, S] as 4 tiles of [128, S] each
            xh, xl = [], []
            for ct in range(CT_C):
                th = xp.tile([P, S], F32R, tag="xs")
                tl = xp.tile([P, S], F32R, tag="xs")
                nc.sync.dma_start(out=th[:], in_=xt_hi[ct * P:(ct + 1) * P, :])
                nc.sync.dma_start(out=tl[:], in_=xt_lo[ct * P:(ct + 1) * P, :])
                xh.append(th)
                xl.append(tl)

            # ---------- helpers ----------
            def bn_affine(gstats, n_ct, g_sl, b_sl):
                """gstats [128, 2*n_ct] = [sums | sumsqs] (global).
                Returns (a_sc, c_sc) [128, 4*n_ct]: per-t-scaled affine."""
                mean = bnp.tile([P, n_ct], F32, tag="mean")
                var = bnp.tile([P, n_ct], F32, tag="var")
                tmpb = bnp.tile([P, n_ct], F32, tag="btmp")
                nc.vector.tensor_scalar(mean[:], gstats[:, 0:n_ct],
                                        1.0 / S_TOT, None, AX.mult)
                nc.vector.tensor_scalar(var[:], gstats[:, n_ct:2 * n_ct],
                                        1.0 / S_TOT, None, AX.mult)
                nc.vector.tensor_mul(tmpb[:], mean[:], mean[:])
                nc.vector.tensor_tensor(out=var[:], in0=var[:], in1=tmpb[:],
                                        op=AX.subtract)
                nc.scalar.activation(var[:], var[:], AF.Sqrt, bias=eps_t[:])
                nc.vector.reciprocal(var[:], var[:])
                a0 = bnp.tile([P, n_ct], F32, tag="a0")
                c0 = bnp.tile([P, n_ct], F32, tag="c0")
                nc.vector.tensor_mul(a0[:], var[:], g_sl)
                nc.vector.tensor_mul(tmpb[:], mean[:], a0[:])
                nc.vector.tensor_tensor(out=c0[:], in0=b_sl, in1=tmpb[:],
                                        op=AX.subtract)
                a_sc = bnp.tile([P, 4 * n_ct], F32, tag="asc")
                c_sc = bnp.tile([P, 4 * n_ct], F32, tag="csc")
                for t in range(T):
                    s = float(2.0 ** (t - 1))
                    nc.vector.tensor_scalar(a_sc[:, t * n_ct:(t + 1) * n_ct],
                                            a0[:], s, None, AX.mult)
                    nc.vector.tensor_scalar(c_sc[:, t * n_ct:(t + 1) * n_ct],
                                            c0[:], s, None, AX.mult)
                return a_sc, c_sc

            def lif_tile(h_t, ct, n_ct, a_sc, c_sc, spk_writer):
                """LIF over the 4 time slabs of h_t [128, S].
                a_sc/c_sc None -> raw input, scale 2^(t-1) (o-lif).
                spk_writer(t, m_ap) emits the spike tensor for slab t."""
                m = mp.tile([P, SLAB], F32, tag="m")
                for t in range(T):
                    sl = h_t[:, t * SLAB:(t + 1) * SLAB]
                    thr = float(2.0 ** t)
                    if a_sc is None:
                        sa = float(2.0 ** (t - 1))
                        if t == 0:
                            nc.vector.tensor_scalar(m[:], sl, sa, None,
                                                    AX.mult)
                        else:
                            nc.vector.affine_then_add(m[:], sl, m[:], sa, 0.0)
                    else:
                        sa = a_sc[:, t * n_ct + ct:t * n_ct + ct + 1]
                        sc = c_sc[:, t * n_ct + ct:t * n_ct + ct + 1]
                        if t == 0:
                            nc.vector.tensor_scalar(m[:], sl, sa, sc,
                                                    AX.mult, AX.add)
                        else:
                            nc.vector.affine_then_add(m[:], sl, m[:], sa, sc)
                    spk_writer(t, m[:])
                    if t < T - 1:
                        nc.vector._custom_dve(TENSOR_MASK, out=m[:], in0=m[:],
                                              in1=m[:], s0=thr, imm2=0.0)

            def allreduce_stats(arin_sb, width):
                ain = dramp.tile([P, width], F32, tag="arin")
                aout = dramp.tile([P, width], F32, tag="arout", addr_space="Shared")
                nc.sync.dma_start(out=ain[:], in_=arin_sb[:])
                if NO_COLL:
                    nc.sync.dma_start(out=aout[:], in_=ain[:])
                else:
                    nc.gpsimd.collective_compute(
                        "AllReduce", AX.add, replica_groups=RG,
                        ins=[ain.opt()], outs=[aout.opt()])
                g = statp.tile([P, width], F32, tag="gstats")
                nc.sync.dma_start(out=g[:], in_=aout[:])
                return g

            def mm_layer_1pass(w_tiles, rhs_fn, psum_pool, n_ct, npass=1):
                """Generic [n_ct tiles] x [4 slabs] matmul with stats.
                rhs_fn(pass_i, kc, ts) -> AP [128, SLAB]; w_tiles[(pass_i,
                kc, ct)] -> lhsT AP [128, 128]. Returns (h_tiles, arin)."""
                n_kc = len({k[1] for k in w_tiles}) if isinstance(w_tiles, dict) else CT_C
                arin = statp.tile([P, 2 * n_ct], F32, tag="arin_sb")
                sumstage = statp.tile([P, n_ct * 4], F32, tag="sumstage")
                h_tiles = []
                sqstage = statp.tile([P, n_ct * 4], F32, tag="sqstage")
                for ct in range(n_ct):
                    h_t = hp.tile([P, S], F32, tag="h")
                    h_tiles.append(h_t)
                    for ts in range(T):
                        ps = psum_pool.tile([P, SLAB], F32, tag="ps")
                        first = True
                        for pi in range(npass):
                            for kc in range(n_kc):
                                nc.tensor.matmul(
                                    ps[:], w_tiles[(pi, kc, ct)],
                                    rhs_fn(pi, kc, ts),
                                    start=first,
                                    stop=(pi == npass - 1 and kc == n_kc - 1))
                                first = False
                        nc.scalar.activation(
                            h_t[:, ts * SLAB:(ts + 1) * SLAB], ps[:], AF.Copy,
                            accum_out=sumstage[:, ct * 4 + ts:ct * 4 + ts + 1])
                        scr = scrp.tile([P, SLAB], F32, tag="scr", name="scr")
                        nc.scalar.activation(
                            scr[:], ps[:], AF.Square,
                            accum_out=sqstage[:, ct * 4 + ts:ct * 4 + ts + 1])
                nc.vector.tensor_reduce(
                    arin[:, 0:n_ct],
                    sumstage[:].rearrange("p (c t) -> p c t", t=4),
                    axis=mybir.AxisListType.X, op=AX.add)
                nc.vector.tensor_reduce(
                    arin[:, n_ct:2 * n_ct],
                    sqstage[:].rearrange("p (c t) -> p c t", t=4),
                    axis=mybir.AxisListType.X, op=AX.add)
                return h_tiles, arin

            # =======================================================
            # Phases 1-4: qkv (1-pass f32r) / attention / o-lif / p
            # =======================================================
            with ExitStack() as _es1:
                wqp = _es1.enter_context(tc.tile_pool(name="wqkv", bufs=5))
                spkp = _es1.enter_context(tc.tile_pool(name="spkbf", bufs=12))
                psmm = _es1.enter_context(tc.tile_pool(name="psmm", bufs=2, space="PSUM"))

                spk_qkv = {}
                for nm in ("q", "k", "v"):
                    wsb = []
                    for kc in range(CT_C):
                        w_sb = wqp.tile([P, C], F32R, tag="wq")
                        nc.sync.dma_start(
                            out=w_sb[:], in_=w_lin[nm][kc * P:(kc + 1) * P, :])
                        wsb.append(w_sb)
                    wt = {(0, kc, ct): wsb[kc][:, ct * P:(ct + 1) * P]
                          for kc in range(CT_C) for ct in range(CT_C)}
                    h_tiles, arin = mm_layer_1pass(
                        wt, lambda pi, kc, ts: xh[kc][:, ts * SLAB:
                                                      (ts + 1) * SLAB],
                        psmm, CT_C)
                    gst = allreduce_stats(arin, 2 * CT_C)
                    g_t, b_t = gbe_sb[nm]
                    a_sc, c_sc = bn_affine(gst, CT_C, g_t[:], b_t[:])
                    tiles = []
                    for ct in range(CT_C):
                        s_t = spkp.tile([P, S], BF16, tag="spk")
                        tiles.append(s_t)

                        def wr(t, m_ap, s_t=s_t):
                            nc.vector.tensor_scalar(
                                s_t[:, t * SLAB:(t + 1) * SLAB], m_ap,
                                float(2.0 ** t), None, AX.is_gt)

                        lif_tile(h_tiles[ct], ct, CT_C, a_sc, c_sc, wr)
                    spk_qkv[nm] = tiles

                # ---- attention ----
                o_tiles = [hp.tile([P, S], F32, tag="h", name="o_t") for _ in range(CT_C)]
                with ExitStack() as _es2:
                    atp = _es2.enter_context(tc.tile_pool(name="attn", bufs=3))
                    kvp = _es2.enter_context(tc.tile_pool(name="kvp", bufs=4))
                    pst = _es2.enter_context(tc.tile_pool(name="pst", bufs=2, space="PSUM"))
                    pskv = _es2.enter_context(tc.tile_pool(name="pskv", bufs=2, space="PSUM"))
                    pso = _es2.enter_context(tc.tile_pool(name="pso", bufs=2, space="PSUM"))
                    for tb in range(T * BL if UPTO >= 2 else 0):
                        base = tb * N
                        kT = [atp.tile([P, C], BF16, tag="kT", name="kT")
                              for _ in range(2)]
                        vT = [atp.tile([P, C], BF16, tag="vT", name="vT")
                              for _ in range(2)]
                        for ct in range(CT_C):
                            for hf in range(2):
                                for src, dst in ((spk_qkv["k"][ct], kT[hf]),
                                                 (spk_qkv["v"][ct], vT[hf])):
                                    pt = pst.tile([P, P], BF16, tag="pt")
                                    nc.tensor.transpose(
                                        pt[:],
                                        src[:, base + hf * P:
                                            base + (hf + 1) * P],
                                        ident[:])
                                    nc.scalar.copy(
                                        dst[:, ct * P:(ct + 1) * P], pt[:])
                        for ct in range(CT_C):
                            po = pso.tile([P, N], F32, tag="po")
                            for hf in range(2):
                                hd = ct * 2 + hf
                                cols = slice(hd * 64, (hd + 1) * 64)
                                prt = slice(hf * 64, (hf + 1) * 64)
                                pkv = pskv.tile([P, 64], F32, tag="pkv")
                                nc.tensor.matmul(pkv[prt, :], kT[0][:, cols],
                                                 vT[0][:, cols],
                                                 start=True, stop=False)
                                nc.tensor.matmul(pkv[prt, :], kT[1][:, cols],
                                                 vT[1][:, cols],
                                                 start=False, stop=True)
                                kv_sb = kvp.tile([P, 64], BF16, tag="kv")
                                nc.scalar.mul(kv_sb[prt, :], pkv[prt, :],
                                              SCALE)
                                nc.tensor.matmul(
                                    po[prt, :], kv_sb[prt, :],
                                    spk_qkv["q"][ct][prt, base:base + N],
                                    start=True, stop=True)
                            nc.scalar.copy(
                                o_tiles[ct][:].bitcast(F32R)[:, base:base + N],
                                po[:])

                # ---- o-lif: spikes f32r in-place ----
                if UPTO >= 3:
                    for ct in range(CT_C):
                        o_r = o_tiles[ct][:].bitcast(F32R)

                        def wr_o(t, m_ap, o_r=o_r):
                            nc.vector.tensor_scalar(
                                o_r[:, t * SLAB:(t + 1) * SLAB], m_ap,
                                float(2.0 ** t), None, AX.is_gt)

                        lif_tile(o_tiles[ct], ct, CT_C, None, None, wr_o)

                # ---- p projection (1-pass f32r on binary spikes) ----
                if UPTO >= 4:
                    wsb = []
                    for kc in range(CT_C):
                        w_sb = wqp.tile([P, C], F32R, tag="wq")
                        nc.sync.dma_start(out=w_sb[:],
                                          in_=w_lin["p"][kc * P:(kc + 1) * P, :])
                        wsb.append(w_sb)
                    wt = {(0, kc, ct): wsb[kc][:, ct * P:(ct + 1) * P]
                          for kc in range(CT_C) for ct in range(CT_C)}
                    hp_tiles, arin_p = mm_layer_1pass(
                        wt,
                        lambda pi, kc, ts: o_tiles[kc][:].bitcast(F32R)[
                            :, ts * SLAB:(ts + 1) * SLAB],
                        psmm, CT_C)
                    gst_p = allreduce_stats(arin_p, 2 * CT_C)
                    g_t, b_t = gbe_sb["p"]
                    a_sc, c_sc = bn_affine(gst_p, CT_C, g_t[:], b_t[:])
                    for ct in range(CT_C):
                        h_t = hp_tiles[ct]

                        def wr_p(t, m_ap, h_t=h_t):
                            nc.vector.tensor_scalar(
                                h_t[:, t * SLAB:(t + 1) * SLAB], m_ap,
                                float(2.0 ** t), None, AX.is_gt)

                        lif_tile(h_t, ct, CT_C, a_sc, c_sc, wr_p)

            # =======================================================
            # Phase 5: x_res = x + spk_p; f32r hi/lo split (reuses xsplit)
            # =======================================================
            with tc.tile_pool(name="tmp", bufs=2) as tmpp:
                if UPTO >= 5:
                    xrh, xrl = [], []
                    for ct in range(CT_C):
                        tmp = tmpp.tile([P, S], F32, tag="tmp")
                        nc.vector.tensor_add(tmp[:], xh[ct][:], xl[ct][:])
                        nc.vector.tensor_add(tmp[:], tmp[:], hp_tiles[ct][:])
                        th = xp.tile([P, S], F32R, tag="xs")
                        tl = xp.tile([P, S], F32R, tag="xs")
                        nc.vector.tensor_copy(th[:], tmp[:])
                        nc.vector.tensor_tensor(out=tl[:], in0=tmp[:], in1=th[:],
                                                op=AX.subtract)
                        xrh.append(th)
                        xrl.append(tl)

                # =====================================================
                # Phase 6: f1 (3-pass f32r splits), two AllReduce halves
                # =====================================================
                if UPTO >= 6:
                    spk_f1_d = dramp.tile([CT_H, P, S], BF16, tag="spkf1")
                    with ExitStack() as _es3:
                        wsp = _es3.enter_context(tc.tile_pool(name="wsm", bufs=9))
                        stgp = _es3.enter_context(tc.tile_pool(name="spkstg", bufs=2))
                        psm2 = _es3.enter_context(tc.tile_pool(name="psmm2", bufs=6, space="PSUM"))
                        xr = {0: xrh, 1: xrl}
                        for half in range(2):
                            wt = {}
                            for wi, w_d in ((0, w_f1h), (1, w_f1l)):
                                for kc in range(CT_C):
                                    w_sb = wsp.tile([P, 8 * P], F32R,
                                                    tag="wf1")
                                    nc.sync.dma_start(
                                        out=w_sb[:],
                                        in_=w_d[kc * P:(kc + 1) * P,
                                                half * 8 * P:
                                                (half + 1) * 8 * P])
                                    for cl in range(8):
                                        wt[(wi, kc, cl)] = \
                                            w_sb[:, cl * P:(cl + 1) * P]
                            # passes: (w_hi, xr_hi), (w_lo, xr_hi), (w_hi, xr_lo)
                            pw = {(0, kc, cl): wt[(0, kc, cl)]
                                  for kc in range(CT_C) for cl in range(8)}
                            pw.update({(1, kc, cl): wt[(1, kc, cl)]
                                       for kc in range(CT_C) for cl in range(8)})
                            pw.update({(2, kc, cl): wt[(0, kc, cl)]
                                       for kc in range(CT_C) for cl in range(8)})
                            rhs_map = {0: xrh, 1: xrh, 2: xrl}
                            h_tiles, arin_f = mm_layer_1pass(
                                pw,
                                lambda pi, kc, ts: rhs_map[pi][kc][
                                    :, ts * SLAB:(ts + 1) * SLAB],
                                psm2, 8, npass=3)
                            gst_f = allreduce_stats(arin_f, 16)
                            g_t, b_t = gbe_sb["f1"]
                            a_sc, c_sc = bn_affine(
                                gst_f, 8, g_t[:, half * 8:half * 8 + 8],
                                b_t[:, half * 8:half * 8 + 8])
                            for cl in range(8):
                                ct = half * 8 + cl
                                stg = stgp.tile([P, S], BF16, tag="stg")

                                def wr_f1(t, m_ap, stg=stg):
                                    nc.vector.tensor_scalar(
                                        stg[:, t * SLAB:(t + 1) * SLAB], m_ap,
                                        float(2.0 ** t), None, AX.is_gt)

                                lif_tile(h_tiles[cl], cl, 8, a_sc, c_sc, wr_f1)
                                nc.sync.dma_start(out=spk_f1_d[ct], in_=stg[:])

                # =====================================================
                # Phase 7: f2 (2-pass bf16), spikes streamed from DRAM
                # =====================================================
                if UPTO >= 7:
                    with ExitStack() as _es4:
                        wf2p = _es4.enter_context(tc.tile_pool(name="wf2p", bufs=32))
                        wtiles = {}
                        for wi, w_d in ((0, w_f2h), (1, w_f2l)):
                            for kc in range(CT_H):
                                w_sb = wf2p.tile([P, C], BF16, tag="wf2")
                                nc.sync.dma_start(
                                    out=w_sb[:],
                                    in_=w_d[kc * P:(kc + 1) * P, :])
                                for ct in range(CT_C):
                                    wtiles[(wi, kc, ct)] = \
                                        w_sb[:, ct * P:(ct + 1) * P]
                        spksp = _es4.enter_context(tc.tile_pool(name="spks", bufs=18))
                        psm3 = _es4.enter_context(tc.tile_pool(name="psmm3", bufs=6, space="PSUM"))
                        h2_tiles = [hp.tile([P, S], F32, tag="h", name="h2_t")
                                    for _ in range(CT_C)]
                        arin_2 = statp.tile([P, 8], F32, tag="arin_sb")
                        sumstage = statp.tile([P, CT_C * 4], F32, tag="sumstage")
                        sqstage2 = statp.tile([P, CT_C * 4], F32, tag="sqstage")
                        for ts in range(T):
                            spks = []
                            for kc in range(CT_H):
                                s_sb = spksp.tile([P, SLAB], BF16, tag="spks")
                                nc.sync.dma_start(
                                    out=s_sb[:],
                                    in_=spk_f1_d[kc][:, ts * SLAB:(ts + 1) * SLAB])
                                spks.append(s_sb)
                            for ct in range(CT_C):
                                ps = psm3.tile([P, SLAB], F32, tag="ps3")
                                first = True
                                for wi in range(2):
                                    for kc in range(CT_H):
                                        nc.tensor.matmul(
                                            ps[:], wtiles[(wi, kc, ct)][:],
                                            spks[kc][:],
                                            start=first,
                                            stop=(wi == 1 and kc == CT_H - 1))
                                        first = False
                                nc.scalar.activation(
                                    h2_tiles[ct][:, ts * SLAB:(ts + 1) * SLAB],
                                    ps[:], AF.Copy,
                                    accum_out=sumstage[:, ct * 4 + ts:
                                                       ct * 4 + ts + 1])
                                scr = scrp.tile([P, SLAB], F32, tag="scr",
                                                name="scr")
                                nc.scalar.activation(
                                    scr[:], ps[:], AF.Square,
                                    accum_out=sqstage2[:, ct * 4 + ts:
                                                        ct * 4 + ts + 1])
                        nc.vector.tensor_reduce(
                            arin_2[:, 0:CT_C],
                            sumstage[:].rearrange("p (c t) -> p c t", t=4),
                            axis=mybir.AxisListType.X, op=AX.add)
                        nc.vector.tensor_reduce(
                            arin_2[:, CT_C:2 * CT_C],
                            sqstage2[:].rearrange("p (c t) -> p c t", t=4),
                            axis=mybir.AxisListType.X, op=AX.add)
                        gst_2 = allreduce_stats(arin_2, 8)
                        g_t, b_t = gbe_sb["f2"]
                        a_sc, c_sc = bn_affine(gst_2, CT_C, g_t[:], b_t[:])
                        for ct in range(CT_C):
                            h_t = h2_tiles[ct]

                            def wr_f2(t, m_ap, h_t=h_t):
                                nc.vector.tensor_scalar(
                                    h_t[:, t * SLAB:(t + 1) * SLAB], m_ap,
                                    float(2.0 ** t), None, AX.is_gt)

                            lif_tile(h_t, ct, CT_C, a_sc, c_sc, wr_f2)

                # =====================================================
                # Phase 8: out = x_res + spk_f2  (= xrh + xrl + spk)
                # =====================================================
                if UPTO >= 8:
                    for ct in range(CT_C):
                        tmp = tmpp.tile([P, S], F32, tag="tmp")
                        nc.vector.tensor_add(tmp[:], xrh[ct][:], xrl[ct][:])
                        nc.vector.tensor_add(tmp[:], tmp[:], h2_tiles[ct][:])
                        nc.sync.dma_start(out=out_d[ct * P:(ct + 1) * P, :],
                                          in_=tmp[:])

    nc.compile()
    return nc


def _get_nc():
    if "nc" not in _CACHE:
        _CACHE["nc"] = _build()
    return _CACHE["nc"]


def _make_in_maps(inputs):
    x = np.asarray(inputs["x"], np.float32)
    base = {}
    for nm in ("q", "k", "v", "p"):
        base[f"w_{nm}"] = _round_mant(np.asarray(inputs[f"{nm}_W"],
                                                 np.float32))
    base["w_f1h"], base["w_f1l"] = _f32r_split(np.asarray(inputs["f1_W"],
                                                          np.float32))
    base["w_f2h"], base["w_f2l"] = _bf16_split(np.asarray(inputs["f2_W"],
                                                          np.float32))
    for nm, n_ct in (("q", CT_C), ("k", CT_C), ("v", CT_C), ("p", CT_C),
                     ("f1", CT_H), ("f2", CT_C)):
        base[f"{nm}_gp"] = _pack_ch(inputs[f"{nm}_g"], n_ct)
        base[f"{nm}_bp"] = _pack_ch(inputs[f"{nm}_be"], n_ct)
    in_maps = []
    for c in range(NCORES):
        xs = x[:, c * BL:(c + 1) * BL].reshape(S, C)
        hi, lo = _f32r_split(np.ascontiguousarray(xs.T))
        m = dict(base)
        m["xt_hi"] = hi
        m["xt_lo"] = lo
        in_maps.append(m)
    return in_maps


def kernel(**inputs):
    in_maps = _make_in_maps(inputs)
    nc = _get_nc()
    res = bass_utils.run_bass_kernel_spmd(nc, in_maps,
                                          core_ids=list(range(NCORES)))
    _CACHE["last_results"] = res

    out = np.empty((T, B, N, C), np.float32)
    for c in range(NCORES):
        oc = np.asarray(res.results[c]["outT"])   # [C, S]
        out[:, c * BL:(c + 1) * BL] = oc.T.reshape(T, BL, N, C)
    return out

